# revision 12
# baseline (speedup 1.0000x reference)
"""Bahdanau-style attention kernel for Trainium2 (8 NeuronCores, data-parallel).

Computes, for each batch b:
    h_proj = hidden @ w_h^T + attn_b                  # [H]
    e_proj = enc[b] @ w_e^T                           # [L, H]
    energy = tanh(h_proj + e_proj)                    # [L, H]
    scores = energy @ v_w                             # [L]
    weights = softmax(scores)                         # [L]
    context[b] = weights @ enc[b]                     # [H]

Sharding: data-parallel over batch B=32 across 8 cores (4 batches/core).
Params are replicated. The softmax max-subtraction is skipped (scores are
bounded by sum|v| <= 32, exp is safe in fp32); the 1/Z normalization is
folded into the final context scaling.

Matmuls use the float32r dtype (fp32 data, single-pass PE mode, 1 cycle/row
at free-dim >= 256 - same speed as bf16 at much better precision).

Built on bacc.Bacc so compile() runs the TRN2 wait-splitting passes
(move_matmul_waits_to_ldweights / generate_event_semaphores).
"""

import numpy as np

H = 1024
B = 32
L = 2048
NCORES = 8
BPC = B // NCORES          # batches per core = 4
KC = H // 128              # contraction chunks = 8
OC = H // 128              # output-feature chunks = 8
NLT = L // 512             # l-tiles of 512 = 4
NLCH = L // 128            # l-chunks of 128 = 16

_CACHED_NC = None


def _build_kernel():
    from contextlib import ExitStack

    import concourse.tile as tile
    from concourse import bacc
    from concourse import mybir
    from concourse.masks import make_identity

    f32 = mybir.dt.float32
    f32r = mybir.dt.float32r
    AF = mybir.ActivationFunctionType

    nc = bacc.Bacc("TRN2", target_bir_lowering=False, debug=False,
                   num_devices=NCORES)

    encT = nc.dram_tensor("encT", [BPC, H, L], f32, kind="ExternalInput").ap()
    encN = nc.dram_tensor("encN", [BPC, L, H], f32, kind="ExternalInput").ap()
    w_eT = nc.dram_tensor("w_eT", [H, H], f32, kind="ExternalInput").ap()
    w_hT = nc.dram_tensor("w_hT", [H, H], f32, kind="ExternalInput").ap()
    hidT = nc.dram_tensor("hidT", [H, BPC], f32, kind="ExternalInput").ap()
    b_in = nc.dram_tensor("attn_b", [H], f32, kind="ExternalInput").ap()
    v_in = nc.dram_tensor("v_w", [H], f32, kind="ExternalInput").ap()
    ctx_out = nc.dram_tensor("ctx", [BPC, H], f32, kind="ExternalOutput").ap()
    # DRAM bounce buffer used to transpose exp(scores) [1,512] -> [128,4]
    escr = nc.dram_tensor("escr", [BPC, L], f32).ap()

    with tile.TileContext(nc) as tc, ExitStack() as ctx:
        consts = ctx.enter_context(tc.tile_pool(name="consts", bufs=1))
        wh_pool = ctx.enter_context(tc.tile_pool(name="wh", bufs=2))
        encT_pool = ctx.enter_context(tc.tile_pool(name="encT", bufs=3))
        encN_pool = ctx.enter_context(tc.tile_pool(name="encN", bufs=2))
        en_pool = ctx.enter_context(tc.tile_pool(name="energy", bufs=4))
        small = ctx.enter_context(tc.tile_pool(name="small", bufs=2))
        expwT_pool = ctx.enter_context(tc.tile_pool(name="expwT", bufs=2))

        # ---- constants ----
        we_sb = consts.tile([128, KC, H], f32r)          # w_e^T  [h-part, k, o]
        nc.sync.dma_start(
            out=we_sb,
            in_=w_eT.rearrange("(k p) o -> p k o", p=128).bitcast(f32r))
        b_sb = consts.tile([128, KC], f32)              # attn_b chunks
        nc.sync.dma_start(out=b_sb, in_=b_in.rearrange("(k p) -> p k", p=128))
        v_sb = consts.tile([128, OC], f32r)              # v_w chunks
        nc.sync.dma_start(
            out=v_sb, in_=v_in.rearrange("(k p) -> p k", p=128).bitcast(f32r))
        hidT_sb = consts.tile([128, KC, BPC], f32r)      # hidden^T chunks
        nc.sync.dma_start(
            out=hidT_sb,
            in_=hidT.rearrange("(k p) b -> p k b", p=128).bitcast(f32r))
        ident = consts.tile([128, 128], f32)
        make_identity(nc, ident)
        ones_f32 = consts.tile([128, 1], f32)
        nc.vector.memset(ones_f32, 1.0)
        ones_sb = consts.tile([128, 1], f32r)
        nc.vector.tensor_copy(ones_sb, ones_f32)

        # ---- h_projT = (hidden @ w_h^T)^T + attn_b  -> [128, OC, BPC] ----
        hproj_nat = consts.tile([BPC, H], f32)          # natural [b, o]
        hproj_sb = consts.tile([128, OC, BPC], f32)     # transposed + bias
        with tc.tile_pool(name="pp_pro", bufs=1, space="PSUM") as pp_pro:
            for half in range(2):
                ph = pp_pro.tile([BPC, 512], f32, tag="ph")
                for k in range(KC):
                    wh_sb = wh_pool.tile([128, 512], f32r, tag="wh")
                    nc.sync.dma_start(
                        out=wh_sb,
                        in_=w_hT[k * 128:(k + 1) * 128,
                                 half * 512:(half + 1) * 512]
                        .bitcast(f32r))
                    nc.tensor.matmul(
                        ph,
                        hidT_sb[:, k, :],
                        wh_sb,
                        start=(k == 0), stop=(k == KC - 1),
                    )
                nc.vector.tensor_copy(
                    hproj_nat[:, half * 512:(half + 1) * 512], ph)
            for o in range(OC):
                pt2 = pp_pro.tile([128, BPC], f32, tag="pt2")
                nc.tensor.transpose(pt2, hproj_nat[:, o * 128:(o + 1) * 128],
                                    ident[0:BPC, 0:BPC])
                nc.scalar.activation(hproj_sb[:, o, :], pt2, AF.Identity,
                                     bias=b_sb[:, o:o + 1])

        pp_e = ctx.enter_context(tc.tile_pool(name="pp_e", bufs=2, space="PSUM"))
        pp_s = ctx.enter_context(tc.tile_pool(name="pp_s", bufs=2, space="PSUM"))
        pp_c = ctx.enter_context(tc.tile_pool(name="pp_c", bufs=2, space="PSUM"))
        pp_t = ctx.enter_context(tc.tile_pool(name="pp_t", bufs=1, space="PSUM"))

        # ---- main per-batch pipeline ----
        for b in range(BPC):
            expwT = expwT_pool.tile([128, NLCH], f32r, tag="expwT")

            # pass A: scores for this batch
            for lt in range(NLT):
                encTs = encT_pool.tile([128, KC, 512], f32r, tag="encTs")
                nc.sync.dma_start(
                    out=encTs,
                    in_=encT[b].rearrange("(k p) l -> p k l", p=128)
                    [:, :, lt * 512:(lt + 1) * 512].bitcast(f32r),
                )
                psum_sc = pp_s.tile([1, 512], f32, tag="psc")
                for o in range(OC):
                    pe = pp_e.tile([128, 512], f32, tag="pe")
                    for k in range(KC):
                        nc.tensor.matmul(
                            pe,
                            we_sb[:, k, o * 128:(o + 1) * 128],
                            encTs[:, k, :],
                            start=(k == 0), stop=(k == KC - 1),
                        )
                    en = en_pool.tile([128, 512], f32r, tag="en")
                    nc.scalar.activation(en, pe, AF.Tanh,
                                         bias=hproj_sb[:, o, b:b + 1])
                    nc.tensor.matmul(psum_sc, v_sb[:, o:o + 1], en,
                                     start=(o == 0), stop=(o == OC - 1))
                # exp (no max subtraction; scores bounded by sum|v| <= 32)
                expw = small.tile([1, 512], f32, tag="expw")
                nc.scalar.activation(expw, psum_sc, AF.Exp)
                # transpose exp(scores) into [l-part, chunk] layout via DRAM
                nc.sync.dma_start(
                    out=escr[b:b + 1, lt * 512:(lt + 1) * 512], in_=expw)
                nc.sync.dma_start(
                    out=expwT[:, lt * 4:(lt + 1) * 4],
                    in_=escr[b, lt * 512:(lt + 1) * 512]
                    .rearrange("(c p) -> p c", p=128).bitcast(f32r),
                )

            # Z = sum(exp(scores)) via ones-matmul + free-dim reduce
            pz = pp_t.tile([1, NLCH], f32, tag="pz")
            nc.tensor.matmul(pz, ones_sb, expwT, start=True, stop=True)
            zs = small.tile([1, 1], f32, tag="zs")
            nc.vector.reduce_sum(zs, pz, axis=mybir.AxisListType.X)
            rz = small.tile([1, 1], f32, tag="rz")
            nc.vector.reciprocal(rz, zs)

            # pass B: context = exp(scores)^T @ enc_natural, scaled by 1/Z
            pcs = [pp_c.tile([1, 512], f32, tag="pc", name=f"pc{i}")
                   for i in range(2)]
            for g in range(4):
                encNs = encN_pool.tile([128, 4, H], f32r, tag="encNs")
                nc.sync.dma_start(
                    out=encNs,
                    in_=encN[b, g * 512:(g + 1) * 512, :]
                    .rearrange("(j p) h -> p j h", p=128).bitcast(f32r),
                )
                for j in range(4):
                    lc = g * 4 + j
                    for half in range(2):
                        nc.tensor.matmul(
                            pcs[half],
                            expwT[:, lc:lc + 1],
                            encNs[:, j, half * 512:(half + 1) * 512],
                            start=(lc == 0), stop=(lc == NLCH - 1),
                        )
            ctx_sb = small.tile([1, H], f32, tag="ctx")
            for half in range(2):
                nc.vector.tensor_scalar_mul(
                    ctx_sb[:, half * 512:(half + 1) * 512], pcs[half], rz)
            nc.sync.dma_start(out=ctx_out[b:b + 1, :], in_=ctx_sb)

    nc.compile()
    return nc


def _get_nc():
    global _CACHED_NC
    if _CACHED_NC is None:
        _CACHED_NC = _build_kernel()
    return _CACHED_NC


def kernel(hidden, encoder_outputs, attn_w, attn_b, v_w):
    from concourse.bass_utils import run_bass_kernel_spmd

    hidden = np.asarray(hidden, dtype=np.float32)
    encoder_outputs = np.asarray(encoder_outputs, dtype=np.float32)
    attn_w = np.asarray(attn_w, dtype=np.float32)
    attn_b = np.asarray(attn_b, dtype=np.float32)
    v_w = np.asarray(v_w, dtype=np.float32)

    w_eT = np.ascontiguousarray(attn_w[:, H:].T)    # [h_in, o]
    w_hT = np.ascontiguousarray(attn_w[:, :H].T)    # [h_in, o]

    in_maps = []
    for c in range(NCORES):
        sl = slice(c * BPC, (c + 1) * BPC)
        in_maps.append({
            "encT": np.ascontiguousarray(encoder_outputs[sl].transpose(0, 2, 1)),
            "encN": np.ascontiguousarray(encoder_outputs[sl]),
            "w_eT": w_eT,
            "w_hT": w_hT,
            "hidT": np.ascontiguousarray(hidden[sl].T),
            "attn_b": attn_b,
            "v_w": v_w,
        })

    nc = _get_nc()
    res = run_bass_kernel_spmd(nc, in_maps, list(range(NCORES)))
    out = np.concatenate([res.results[c]["ctx"] for c in range(NCORES)], axis=0)
    return out.astype(np.float32)
